# revision 1
# baseline (speedup 1.0000x reference)
"""nn_BlockCirculantLinear on 8 Trainium2 cores (Bass/Tile, float32r).

Math.  The reference computes, per output block o (8 blocks of P=512):
    y_o = sum_i real(IFFT(Lam[o,i] * FFT(x_i * sf_i)))
With x real, this factors exactly into three real linear stages:
  1. forward  : X_i = Fe @ (sf*x)_i^T      -- real-DFT coords, per block i
  2. middle   : Y_o = sum_i M_oi X_i       -- per-frequency 2x2 mixes
  3. inverse  : y_o^T = Fi @ Y_o
Coordinate packing per block: c=0 -> (f=0, re); c=1 -> (f=256, re);
c=2f/2f+1 -> (f, re/im) for f=1..255.  Frequency pair (f, P-f) is folded
into one 2x2 real block using the Hermitian symmetry of X:
  A_f = (l1r+l2r) Xr + (l2i-l1i) Xi ;  B_f = (l1i-l2i) Xr + (l1r+l2r) Xi
where l1 = Lam[o,i,f], l2 = Lam[o,i,P-f], and
  y[t] = (1/P)[A_0 + A_256 (-1)^t + sum_f (A_f cos(2pi f t/P) - B_f sin(..))].
This does 34 GFLOP/core of 128x128-tile matmuls (half of the dense-W
formulation) with only ~18 MiB of transform constants.

Sharding: data-parallel -- 16384 rows split 8 ways; constants replicated.
sign_flip is folded into x on the host; bias is added on the host after
gathering (host also transposes x in / y^T out, which is free input/output
marshalling).

Device kernel (per core): 2048 float32r matmuls of [K=128, M=128, N=512],
PSUM-resident accumulation (fwd K=512 in 4; mid sum over 8 blocks; inv K=512
in 4), psum pools 2/4/2 banks, mid constants streamed with 9-deep prefetch,
outputs evicted via DVE/ACT copies.  Measured ~415 us/core/pass on HW.
"""
import os
from contextlib import ExitStack

import numpy as np

import concourse.mybir as mybir
import concourse.bacc as bacc
import concourse.tile as tile
from concourse.bass_utils import run_bass_kernel_spmd

N_CORES = 8
ROWS = 16384
RPC = ROWS // N_CORES      # 2048 rows per core
F = 4096
P = 512
NBLK = 8
CHUNK = 512                # rows per pipelined chunk (= matmul free dim)
_NC_CACHE = {}

DT = mybir.dt.float32r     # fp32 in memory, FP22 in the PE, fp32 accumulate
DTO = mybir.dt.float32


def build_transforms(spectral_real, spectral_imag, dtype=np.float64):
    """Fe [c, feat], Fi [t, c], M [o, i, c_out, c_in] (2x2 block diagonal)."""
    s = np.arange(P)
    f = np.arange(1, P // 2)
    theta = 2 * np.pi * np.outer(f, s) / P

    Fe = np.zeros((P, P), dtype)
    Fe[0, :] = 1.0
    Fe[1, :] = (-1.0) ** s
    Fe[2::2, :] = np.cos(theta)
    Fe[3::2, :] = -np.sin(theta)

    Fi = np.zeros((P, P), dtype)
    Fi[:, 0] = 1.0 / P
    Fi[:, 1] = ((-1.0) ** s) / P
    Fi[:, 2::2] = np.cos(theta).T / P
    Fi[:, 3::2] = -np.sin(theta).T / P

    lam_r = spectral_real.astype(dtype)
    lam_i = spectral_imag.astype(dtype)
    M = np.zeros((NBLK, NBLK, P, P), dtype)
    M[:, :, 0, 0] = lam_r[:, :, 0]
    M[:, :, 1, 1] = lam_r[:, :, P // 2]
    l1r = lam_r[:, :, 1:P // 2]; l1i = lam_i[:, :, 1:P // 2]
    l2r = lam_r[:, :, :P // 2:-1]; l2i = lam_i[:, :, :P // 2:-1]
    ce = np.arange(2, P, 2); co = ce + 1
    M[:, :, ce, ce] = l1r + l2r
    M[:, :, ce, co] = l2i - l1i
    M[:, :, co, ce] = l1i - l2i
    M[:, :, co, co] = l1r + l2r
    return Fe, Fi, M


def host_transforms(spectral_real, spectral_imag):
    Fe, Fi, M = build_transforms(spectral_real, spectral_imag)
    fwdT = np.ascontiguousarray(Fe.T.astype(np.float32))     # lhsT [feat, c]
    invT = np.ascontiguousarray(Fi.T.astype(np.float32))     # lhsT [c, t]
    # mid lhsT tiles packed per (o, ct): [128, 8 blocks * 128]
    midT = np.zeros((NBLK, 4, 128, NBLK * 128), np.float32)
    for o in range(NBLK):
        for ct in range(4):
            sl = slice(ct * 128, (ct + 1) * 128)
            for i in range(NBLK):
                midT[o, ct, :, i * 128:(i + 1) * 128] = M[o, i, sl, sl].T
    return fwdT, invT, midT


def build_nc(repeat: int = 1):
    key = (CHUNK, repeat)
    if key in _NC_CACHE:
        return _NC_CACHE[key]
    nc = bacc.Bacc("TRN2", target_bir_lowering=False, debug=False,
                   num_devices=N_CORES)
    xT = nc.dram_tensor("xT", [F, RPC], DT, kind="ExternalInput")
    fwdT = nc.dram_tensor("fwdT", [P, P], DT, kind="ExternalInput")
    invT = nc.dram_tensor("invT", [P, P], DT, kind="ExternalInput")
    midT = nc.dram_tensor("midT", [NBLK, 4, 128, NBLK * 128], DT,
                          kind="ExternalInput")
    yT = nc.dram_tensor("yT", [F, RPC], DTO, kind="ExternalOutput")

    n_chunks = RPC // CHUNK

    with tile.TileContext(nc) as tc:
        with ExitStack() as ctx:
            const = ctx.enter_context(tc.tile_pool(name="const", bufs=1))
            fwd_sb = const.tile([128, 16 * 128], DT)
            inv_sb = const.tile([128, 16 * 128], DT)
            for kc in range(4):
                for mt in range(4):
                    j = (kc * 4 + mt) * 128
                    nc.sync.dma_start(fwd_sb[:, j:j + 128],
                                      fwdT[kc * 128:(kc + 1) * 128,
                                           mt * 128:(mt + 1) * 128])
                    nc.sync.dma_start(inv_sb[:, j:j + 128],
                                      invT[kc * 128:(kc + 1) * 128,
                                           mt * 128:(mt + 1) * 128])

            xpool = ctx.enter_context(tc.tile_pool(name="x", bufs=16))
            Xpool = ctx.enter_context(tc.tile_pool(name="X", bufs=34))
            Ypool = ctx.enter_context(tc.tile_pool(name="Y", bufs=10))
            mpool = ctx.enter_context(tc.tile_pool(name="mid", bufs=9))
            opool = ctx.enter_context(tc.tile_pool(name="out", bufs=5))
            psf = ctx.enter_context(tc.tile_pool(name="psf", bufs=2,
                                                 space="PSUM"))
            psm = ctx.enter_context(tc.tile_pool(name="psm", bufs=4,
                                                 space="PSUM"))
            psi = ctx.enter_context(tc.tile_pool(name="psi", bufs=2,
                                                 space="PSUM"))

            def chunk_body(c):
                r0 = c * CHUNK
                x_sb = {}
                for i in range(NBLK):
                    for kc in range(4):
                        t = xpool.tile([128, CHUNK], DT, tag="x", name="xt")
                        nc.sync.dma_start(
                            t[:], xT[(i * 4 + kc) * 128:(i * 4 + kc + 1) * 128,
                                     r0:r0 + CHUNK])
                        x_sb[i, kc] = t
                X_sb = {}
                for i in range(NBLK):
                    for mt in range(4):
                        ps = psf.tile([128, CHUNK], DTO, tag="f", name="fps")
                        for kc in range(4):
                            nc.tensor.matmul(
                                ps[:],
                                fwd_sb[:, (kc * 4 + mt) * 128:(kc * 4 + mt + 1) * 128],
                                x_sb[i, kc][:],
                                start=(kc == 0), stop=(kc == 3))
                        t = Xpool.tile([128, CHUNK], DT, tag="X", name="Xt")
                        nc.any.tensor_copy(out=t[:], in_=ps[:])
                        X_sb[i, mt] = t
                for o in range(NBLK):
                    Y_sb = {}
                    for ct in range(4):
                        m = mpool.tile([128, NBLK * 128], DT, tag="m",
                                       name="mt_")
                        nc.sync.dma_start(m[:], midT[o, ct])
                        ps = psm.tile([128, CHUNK], DTO, tag="m", name="mps")
                        for i in range(NBLK):
                            nc.tensor.matmul(
                                ps[:], m[:, i * 128:(i + 1) * 128],
                                X_sb[i, ct][:],
                                start=(i == 0), stop=(i == NBLK - 1))
                        t = Ypool.tile([128, CHUNK], DT, tag="Y", name="Yt")
                        nc.any.tensor_copy(out=t[:], in_=ps[:])
                        Y_sb[ct] = t
                    for tt in range(4):
                        ps = psi.tile([128, CHUNK], DTO, tag="i", name="ips")
                        for ct in range(4):
                            nc.tensor.matmul(
                                ps[:],
                                inv_sb[:, (ct * 4 + tt) * 128:(ct * 4 + tt + 1) * 128],
                                Y_sb[ct][:],
                                start=(ct == 0), stop=(ct == 3))
                        t = opool.tile([128, CHUNK], DTO, tag="o", name="ot")
                        nc.any.tensor_copy(out=t[:], in_=ps[:])
                        nc.sync.dma_start(
                            yT[(o * 4 + tt) * 128:(o * 4 + tt + 1) * 128,
                               r0:r0 + CHUNK], t[:])

            def body(_=None):
                for c in range(n_chunks):
                    chunk_body(c)

            if repeat == 1:
                body()
            else:
                with tc.For_i(0, repeat, 1) as it:
                    body(it)
    nc.compile()
    _NC_CACHE[key] = nc
    return nc


def make_in_maps(x, spectral_real, spectral_imag, sign_flip):
    fwdT, invT, midT = host_transforms(spectral_real, spectral_imag)
    xs = (x.reshape(-1, F) * sign_flip[None, :].astype(np.float32))
    in_maps = []
    for c in range(N_CORES):
        shard = xs[c * RPC:(c + 1) * RPC]
        in_maps.append({
            "xT": np.ascontiguousarray(shard.T),
            "fwdT": fwdT, "invT": invT, "midT": midT,
        })
    return in_maps


def kernel(x, spectral_real, spectral_imag, sign_flip, bias):
    x = np.asarray(x, np.float32)
    spectral_real = np.asarray(spectral_real, np.float32)
    spectral_imag = np.asarray(spectral_imag, np.float32)
    sign_flip = np.asarray(sign_flip, np.float32)
    bias = np.asarray(bias, np.float32)
    batch_shape = x.shape[:-1]

    in_maps = make_in_maps(x, spectral_real, spectral_imag, sign_flip)
    nc = build_nc()
    res = run_bass_kernel_spmd(nc, in_maps, list(range(N_CORES)))
    y = np.concatenate(
        [np.ascontiguousarray(res.results[c]["yT"].T) for c in range(N_CORES)],
        axis=0)
    y = y + bias[None, :]
    return y.reshape(*batch_shape, F).astype(np.float32)



# revision 8
# speedup vs baseline: 1.1297x; 1.1297x over previous
"""nn_BlockCirculantLinear on 8 Trainium2 cores (Bass/Tile, bf16).

Math.  Per output block o (8 blocks of P=512):
    y_o = sum_i real(IFFT(Lam[o,i] * FFT(x_i * sf_i)))
With x real this factors into three real linear stages:
  1. forward  : X_i = Fe @ (sf*x)_i^T      -- real-DFT coords, per block i
  2. middle   : Y_o = sum_i M_oi X_i       -- per-frequency 2x2 mixes
  3. inverse  : y_o^T = Fi @ Y_o
Coordinate packing per block: c=0 -> (f=0, re); c=1 -> (f=256, re);
c=2f/2f+1 -> (f, re/im) for f=1..255.  Frequency pair (f, P-f) folds into
one 2x2 real block via Hermitian symmetry, so M[o,i] couples only
same-frequency coordinate pairs (2x2 block diagonal).

Key restructure vs the dense-mid formulation ("quarter stacking"): the
mid only couples same-frequency coords, so a 128x128 mid matmul with a
per-block 128-coord tile has 2/128 useful density.  Instead, stack FOUR
blocks x 32 coords per 128-partition tile: T[jj,rq][p=(i4,c32), r].
Then the mid is 2 dense accumulating matmuls per output tile
(2 kk x 16 rq x 2 jj = 64 matmuls/chunk instead of 256), and the dense
mid constant shrinks from 16.8 MiB streamed every chunk (67 MB/pass of
HBM) to a 2 MiB resident table.  The (re)stacking costs nothing extra:
PSUM evictions become partition-shifted 32-row band copies (DVE/ACT/Pool
lanes can read psum band [32k,32k+32) and write sbuf band [32m,32m+32)),
which replace the full-tile eviction copies the kernel needs anyway.

Per core per pass: (128 fwd + 64 mid + 128 inv) matmuls[K=128,M<=128,
N=512]/chunk x 4 chunks = 1280 matmuls ~= 273us PE; DMA only x+y
(33.5 MB bf16) ~= 95us; eviction copies ~49K cyc/chunk/engine over 3
engines.  All streams bf16 (tolerance 2e-2 >> bf16 error ~6e-3).

Sharding: data-parallel -- 16384 rows split 8 ways; constants replicated.
sign_flip folded into x on host; bias added on host after gathering.
"""
import os
from contextlib import ExitStack

import numpy as np

import concourse.mybir as mybir
import concourse.bacc as bacc
import concourse.tile as tile
from concourse.bass_utils import run_bass_kernel_spmd

N_CORES = 8
ROWS = 16384
RPC = ROWS // N_CORES      # 2048 rows per core
F = 4096
P = 512
NBLK = 8
CHUNK = 512                # rows per pipelined chunk (= matmul free dim)
_NC_CACHE = {}

DT = mybir.dt.bfloat16
NPDT = mybir.dt.np(DT)


def build_transforms(spectral_real, spectral_imag, dtype=np.float64):
    """Fe [c, feat], Fi [t, c], M [o, i, c_out, c_in] (2x2 block diagonal)."""
    s = np.arange(P)
    f = np.arange(1, P // 2)
    theta = 2 * np.pi * np.outer(f, s) / P

    Fe = np.zeros((P, P), dtype)
    Fe[0, :] = 1.0
    Fe[1, :] = (-1.0) ** s
    Fe[2::2, :] = np.cos(theta)
    Fe[3::2, :] = -np.sin(theta)

    Fi = np.zeros((P, P), dtype)
    Fi[:, 0] = 1.0 / P
    Fi[:, 1] = ((-1.0) ** s) / P
    Fi[:, 2::2] = np.cos(theta).T / P
    Fi[:, 3::2] = -np.sin(theta).T / P

    lam_r = spectral_real.astype(dtype)
    lam_i = spectral_imag.astype(dtype)
    M = np.zeros((NBLK, NBLK, P, P), dtype)
    M[:, :, 0, 0] = lam_r[:, :, 0]
    M[:, :, 1, 1] = lam_r[:, :, P // 2]
    l1r = lam_r[:, :, 1:P // 2]; l1i = lam_i[:, :, 1:P // 2]
    l2r = lam_r[:, :, :P // 2:-1]; l2i = lam_i[:, :, :P // 2:-1]
    ce = np.arange(2, P, 2); co = ce + 1
    M[:, :, ce, ce] = l1r + l2r
    M[:, :, ce, co] = l2i - l1i
    M[:, :, co, ce] = l1i - l2i
    M[:, :, co, co] = l1r + l2r
    return Fe, Fi, M


def host_transforms(spectral_real, spectral_imag):
    Fe, Fi, M = build_transforms(spectral_real, spectral_imag)
    fwdT = np.ascontiguousarray(Fe.T).astype(NPDT)           # lhsT [feat, c]
    invT = np.ascontiguousarray(Fi.T).astype(NPDT)           # lhsT [c, t]
    # quarter-stacked mid lhsT per (kk, jj, rq):
    #   lhsT[p_in=(i4,c32), p_out=(o4,c'32)] = M[kk*4+o4, jj*4+i4, c', c]
    midQ = np.zeros((2, 2, 16, 128, 128), np.float64)
    for kk in range(2):
        for jj in range(2):
            for rq in range(16):
                blk = M[kk * 4:(kk + 1) * 4, jj * 4:(jj + 1) * 4,
                        rq * 32:(rq + 1) * 32, rq * 32:(rq + 1) * 32]
                midQ[kk, jj, rq] = (blk.transpose(1, 3, 0, 2)
                                    .reshape(128, 128))
    return fwdT, invT, midQ.reshape(64, 128, 128).astype(NPDT)


def build_nc(repeat: int = 1):
    key = (CHUNK, repeat)
    if key in _NC_CACHE:
        return _NC_CACHE[key]
    nc = bacc.Bacc("TRN2", target_bir_lowering=False, debug=False,
                   num_devices=N_CORES)
    xT = nc.dram_tensor("xT", [F, RPC], DT, kind="ExternalInput")
    fwdT = nc.dram_tensor("fwdT", [P, P], DT, kind="ExternalInput")
    invT = nc.dram_tensor("invT", [P, P], DT, kind="ExternalInput")
    midQ = nc.dram_tensor("midQ", [64, 128, 128], DT, kind="ExternalInput")
    yT = nc.dram_tensor("yT", [F, RPC], DT, kind="ExternalOutput")

    n_chunks = RPC // CHUNK
    F32 = mybir.dt.float32

    with tile.TileContext(nc) as tc:
        with ExitStack() as ctx:
            const = ctx.enter_context(tc.tile_pool(name="const", bufs=1))
            fwd_sb = const.tile([128, 16 * 128], DT)
            inv_sb = const.tile([128, 16 * 128], DT)
            mid_sb = const.tile([128, 64 * 128], DT)
            for kc in range(4):
                for mt in range(4):
                    j = (kc * 4 + mt) * 128
                    nc.sync.dma_start(fwd_sb[:, j:j + 128],
                                      fwdT[kc * 128:(kc + 1) * 128,
                                           mt * 128:(mt + 1) * 128])
                    nc.sync.dma_start(inv_sb[:, j:j + 128],
                                      invT[kc * 128:(kc + 1) * 128,
                                           mt * 128:(mt + 1) * 128])
            for g in range(64):
                nc.sync.dma_start(mid_sb[:, g * 128:(g + 1) * 128],
                                  midQ[g])

            xpool = ctx.enter_context(tc.tile_pool(name="x", bufs=6))
            tpool = ctx.enter_context(tc.tile_pool(name="tq", bufs=2))
            ypool = ctx.enter_context(tc.tile_pool(name="ys", bufs=12))
            opool = ctx.enter_context(tc.tile_pool(name="out", bufs=3))
            psf = ctx.enter_context(tc.tile_pool(name="psf", bufs=3,
                                                 space="PSUM"))
            psm = ctx.enter_context(tc.tile_pool(name="psm", bufs=2,
                                                 space="PSUM"))
            psi = ctx.enter_context(tc.tile_pool(name="psi", bufs=3,
                                                 space="PSUM"))

            def chunk_body(c):
                r0 = c * CHUNK
                # ---- load x (one DMA per pair of input blocks) ----
                x_sb = []
                for h in range(NBLK // 2):
                    t = xpool.tile([128, 8 * CHUNK], DT, tag="x", name="xt")
                    nc.sync.dma_start(
                        t[:].rearrange("p (w r) -> p w r", w=8, r=CHUNK),
                        xT[h * 2 * P:(h + 1) * 2 * P, r0:r0 + CHUNK]
                        .rearrange("(w p) r -> p w r", w=8, p=128))
                    x_sb.append(t)

                def x_slice(i, kc):
                    h, w = divmod(i * 4 + kc, 8)
                    return x_sb[h][:, w * CHUNK:(w + 1) * CHUNK]

                # ---- forward DFT; evict as quarter-stacked tiles ----
                # T[jj, rq] at tall free slice (jj*16+rq); partition
                # (i%4)*32 + c32.  Source psum (i, ct) band rq4 covers
                # rq = ct*4 + rq4.
                tall = tpool.tile([128, 32 * CHUNK], DT, tag="tq",
                                  name="tall")
                for i in range(NBLK):
                    jj = i // 4
                    for ct in range(4):
                        ps = psf.tile([128, CHUNK], F32, tag="f", name="fps")
                        for kc in range(4):
                            nc.tensor.matmul(
                                ps[:],
                                fwd_sb[:, (kc * 4 + ct) * 128:
                                       (kc * 4 + ct + 1) * 128],
                                x_slice(i, kc),
                                start=(kc == 0), stop=(kc == 3))
                        for rq4 in range(4):
                            rq = ct * 4 + rq4
                            nc.any.tensor_copy(
                                out=tall[(i % 4) * 32:(i % 4) * 32 + 32,
                                         (jj * 16 + rq) * CHUNK:
                                         (jj * 16 + rq + 1) * CHUNK],
                                in_=ps[rq4 * 32:(rq4 + 1) * 32, :])

                # ---- mid: 2 dense matmuls per (kk, rq) output tile ----
                # out partitions (o4, c'32); band o4 -> Y_sep[kk*4+o4]
                # partitions rq4*32 + c'32, free ct*CHUNK (rq = ct*4+rq4).
                y_sep = []
                for o in range(NBLK):
                    y_sep.append(ypool.tile([128, 4 * CHUNK], DT, tag="ys",
                                            name="yst"))
                for kk in range(2):
                    for rq in range(16):
                        ct, rq4 = divmod(rq, 4)
                        ps = psm.tile([128, CHUNK], F32, tag="m", name="mps")
                        for jj in range(2):
                            g = (kk * 2 + jj) * 16 + rq
                            nc.tensor.matmul(
                                ps[:], mid_sb[:, g * 128:(g + 1) * 128],
                                tall[:, (jj * 16 + rq) * CHUNK:
                                     (jj * 16 + rq + 1) * CHUNK],
                                start=(jj == 0), stop=(jj == 1))
                        for o4 in range(4):
                            nc.any.tensor_copy(
                                out=y_sep[kk * 4 + o4][
                                    rq4 * 32:(rq4 + 1) * 32,
                                    ct * CHUNK:(ct + 1) * CHUNK],
                                in_=ps[o4 * 32:(o4 + 1) * 32, :])

                # ---- inverse DFT per output block o ----
                for o in range(NBLK):
                    ot = opool.tile([128, 4 * CHUNK], DT, tag="o", name="ot")
                    for tt in range(4):
                        ps = psi.tile([128, CHUNK], F32, tag="i", name="ips")
                        for ct in range(4):
                            nc.tensor.matmul(
                                ps[:],
                                inv_sb[:, (ct * 4 + tt) * 128:
                                       (ct * 4 + tt + 1) * 128],
                                y_sep[o][:, ct * CHUNK:(ct + 1) * CHUNK],
                                start=(ct == 0), stop=(ct == 3))
                        nc.any.tensor_copy(
                            out=ot[:, tt * CHUNK:(tt + 1) * CHUNK], in_=ps[:])
                    nc.sync.dma_start(
                        yT[o * P:(o + 1) * P, r0:r0 + CHUNK]
                        .rearrange("(tt p) r -> p tt r", tt=4, p=128),
                        ot[:].rearrange("p (tt r) -> p tt r", tt=4, r=CHUNK))

            def body(_=None):
                for c in range(n_chunks):
                    chunk_body(c)

            if repeat == 1:
                body()
            else:
                with tc.For_i(0, repeat, 1) as it:
                    body(it)
    nc.compile()
    _NC_CACHE[key] = nc
    return nc


def make_in_maps(x, spectral_real, spectral_imag, sign_flip):
    fwdT, invT, midQ = host_transforms(spectral_real, spectral_imag)
    xs = (x.reshape(-1, F) * sign_flip[None, :].astype(np.float32))
    in_maps = []
    for c in range(N_CORES):
        shard = xs[c * RPC:(c + 1) * RPC]
        in_maps.append({
            "xT": np.ascontiguousarray(shard.T).astype(NPDT),
            "fwdT": fwdT, "invT": invT, "midQ": midQ,
        })
    return in_maps


def kernel(x, spectral_real, spectral_imag, sign_flip, bias):
    x = np.asarray(x, np.float32)
    spectral_real = np.asarray(spectral_real, np.float32)
    spectral_imag = np.asarray(spectral_imag, np.float32)
    sign_flip = np.asarray(sign_flip, np.float32)
    bias = np.asarray(bias, np.float32)
    batch_shape = x.shape[:-1]

    in_maps = make_in_maps(x, spectral_real, spectral_imag, sign_flip)
    nc = build_nc()
    res = run_bass_kernel_spmd(nc, in_maps, list(range(N_CORES)))
    y = np.concatenate(
        [np.ascontiguousarray(np.asarray(res.results[c]["yT"],
                                         dtype=np.float32).T)
         for c in range(N_CORES)],
        axis=0)
    y = y + bias[None, :]
    return y.reshape(*batch_shape, F).astype(np.float32)


# revision 12
# speedup vs baseline: 1.2274x; 1.0865x over previous
"""nn_BlockCirculantLinear on 8 Trainium2 cores (Bass/Tile, bf16).

Math.  Per output block o (8 blocks of P=512):
    y_o = sum_i real(IFFT(Lam[o,i] * FFT(x_i * sf_i)))
With x real this factors into three real linear stages:
  1. forward  : X_i = Fe @ (sf*x)_i^T      -- real-DFT coords, per block i
  2. middle   : Y_o = sum_i M_oi X_i       -- per-frequency 2x2 mixes
  3. inverse  : y_o^T = Fi @ Y_o
Coordinate packing per block: c=0 -> (f=0, re); c=1 -> (f=256, re);
c=2f/2f+1 -> (f, re/im) for f=1..255.  Frequency pair (f, P-f) folds into
one 2x2 real block via Hermitian symmetry, so M[o,i] couples only
same-frequency coordinate pairs (2x2 block diagonal).

Key restructure vs the dense-mid formulation ("quarter stacking"): the
mid only couples same-frequency coords, so a 128x128 mid matmul with a
per-block 128-coord tile has 2/128 useful density.  Instead, stack FOUR
blocks x 32 coords per 128-partition tile: T[jj,rq][p=(i4,c32), r].
Then the mid is 2 dense accumulating matmuls per output tile
(2 kk x 16 rq x 2 jj = 64 matmuls/chunk instead of 256), and the dense
mid constant shrinks from 16.8 MiB streamed every chunk (67 MB/pass of
HBM) to a 2 MiB resident table.  The (re)stacking costs nothing extra:
PSUM evictions become partition-shifted 32-row band copies (DVE/ACT/Pool
lanes can read psum band [32k,32k+32) and write sbuf band [32m,32m+32)),
which replace the full-tile eviction copies the kernel needs anyway.

Per core per pass: (128 fwd + 64 mid + 128 inv) matmuls[K=128,M<=128,
N=512]/chunk x 4 chunks = 1280 matmuls ~= 273us PE; DMA only x+y
(33.5 MB bf16) ~= 95us; eviction copies ~49K cyc/chunk/engine over 3
engines.  All streams bf16 (tolerance 2e-2 >> bf16 error ~6e-3).

Sharding: data-parallel -- 16384 rows split 8 ways; constants replicated.
sign_flip folded into x on host; bias added on host after gathering.
"""
import os
from contextlib import ExitStack

import numpy as np

import concourse.mybir as mybir
import concourse.bacc as bacc
import concourse.tile as tile
from concourse.bass_utils import run_bass_kernel_spmd

N_CORES = 8
ROWS = 16384
RPC = ROWS // N_CORES      # 2048 rows per core
F = 4096
P = 512
NBLK = 8
CHUNK = 512                # rows per pipelined chunk (= matmul free dim)
_NC_CACHE = {}

DT = mybir.dt.bfloat16
NPDT = mybir.dt.np(DT)


def build_transforms(spectral_real, spectral_imag, dtype=np.float64):
    """Fe [c, feat], Fi [t, c], M [o, i, c_out, c_in] (2x2 block diagonal)."""
    s = np.arange(P)
    f = np.arange(1, P // 2)
    theta = 2 * np.pi * np.outer(f, s) / P

    Fe = np.zeros((P, P), dtype)
    Fe[0, :] = 1.0
    Fe[1, :] = (-1.0) ** s
    Fe[2::2, :] = np.cos(theta)
    Fe[3::2, :] = -np.sin(theta)

    Fi = np.zeros((P, P), dtype)
    Fi[:, 0] = 1.0 / P
    Fi[:, 1] = ((-1.0) ** s) / P
    Fi[:, 2::2] = np.cos(theta).T / P
    Fi[:, 3::2] = -np.sin(theta).T / P

    lam_r = spectral_real.astype(dtype)
    lam_i = spectral_imag.astype(dtype)
    M = np.zeros((NBLK, NBLK, P, P), dtype)
    M[:, :, 0, 0] = lam_r[:, :, 0]
    M[:, :, 1, 1] = lam_r[:, :, P // 2]
    l1r = lam_r[:, :, 1:P // 2]; l1i = lam_i[:, :, 1:P // 2]
    l2r = lam_r[:, :, :P // 2:-1]; l2i = lam_i[:, :, :P // 2:-1]
    ce = np.arange(2, P, 2); co = ce + 1
    M[:, :, ce, ce] = l1r + l2r
    M[:, :, ce, co] = l2i - l1i
    M[:, :, co, ce] = l1i - l2i
    M[:, :, co, co] = l1r + l2r
    return Fe, Fi, M


def host_transforms(spectral_real, spectral_imag):
    Fe, Fi, M = build_transforms(spectral_real, spectral_imag)
    fwdT = np.ascontiguousarray(Fe.T).astype(NPDT)           # lhsT [feat, c]
    invT = np.ascontiguousarray(Fi.T).astype(NPDT)           # lhsT [c, t]
    # quarter-stacked mid lhsT per (kk, jj, rq):
    #   lhsT[p_in=(i4,c32), p_out=(o4,c'32)] = M[kk*4+o4, jj*4+i4, c', c]
    midQ = np.zeros((2, 2, 16, 128, 128), np.float64)
    for kk in range(2):
        for jj in range(2):
            for rq in range(16):
                blk = M[kk * 4:(kk + 1) * 4, jj * 4:(jj + 1) * 4,
                        rq * 32:(rq + 1) * 32, rq * 32:(rq + 1) * 32]
                midQ[kk, jj, rq] = (blk.transpose(1, 3, 0, 2)
                                    .reshape(128, 128))
    return fwdT, invT, midQ.reshape(64, 128, 128).astype(NPDT)


def build_nc(repeat: int = 1):
    key = (CHUNK, repeat)
    if key in _NC_CACHE:
        return _NC_CACHE[key]
    nc = bacc.Bacc("TRN2", target_bir_lowering=False, debug=False,
                   num_devices=N_CORES)
    xT = nc.dram_tensor("xT", [F, RPC], DT, kind="ExternalInput")
    fwdT = nc.dram_tensor("fwdT", [P, P], DT, kind="ExternalInput")
    invT = nc.dram_tensor("invT", [P, P], DT, kind="ExternalInput")
    midQ = nc.dram_tensor("midQ", [64, 128, 128], DT, kind="ExternalInput")
    yT = nc.dram_tensor("yT", [F, RPC], DT, kind="ExternalOutput")

    n_chunks = RPC // CHUNK
    F32 = mybir.dt.float32

    with tile.TileContext(nc) as tc:
        with ExitStack() as ctx:
            const = ctx.enter_context(tc.tile_pool(name="const", bufs=1))
            fwd_sb = const.tile([128, 16 * 128], DT)
            inv_sb = const.tile([128, 16 * 128], DT)
            mid_sb = const.tile([128, 64 * 128], DT)
            for kc in range(4):
                for mt in range(4):
                    j = (kc * 4 + mt) * 128
                    nc.sync.dma_start(fwd_sb[:, j:j + 128],
                                      fwdT[kc * 128:(kc + 1) * 128,
                                           mt * 128:(mt + 1) * 128])
                    nc.sync.dma_start(inv_sb[:, j:j + 128],
                                      invT[kc * 128:(kc + 1) * 128,
                                           mt * 128:(mt + 1) * 128])
            for g in range(64):
                nc.sync.dma_start(mid_sb[:, g * 128:(g + 1) * 128],
                                  midQ[g])

            xpool = ctx.enter_context(tc.tile_pool(name="x", bufs=5))
            tpool = ctx.enter_context(tc.tile_pool(name="tq", bufs=1))
            upool = ctx.enter_context(tc.tile_pool(name="us", bufs=2))
            ypool = ctx.enter_context(tc.tile_pool(name="ys", bufs=9))
            opool = ctx.enter_context(tc.tile_pool(name="out", bufs=3))
            psf = ctx.enter_context(tc.tile_pool(name="psf", bufs=2,
                                                 space="PSUM"))
            psm = ctx.enter_context(tc.tile_pool(name="psm", bufs=2,
                                                 space="PSUM"))
            psi = ctx.enter_context(tc.tile_pool(name="psi", bufs=2,
                                                 space="PSUM"))

            def chunk_body(c):
                r0 = c * CHUNK
                # ---- load x (one DMA per pair of input blocks) ----
                x_sb = []
                for h in range(NBLK // 2):
                    t = xpool.tile([128, 8 * CHUNK], DT, tag="x", name="xt")
                    nc.sync.dma_start(
                        t[:].rearrange("p (w r) -> p w r", w=8, r=CHUNK),
                        xT[h * 2 * P:(h + 1) * 2 * P, r0:r0 + CHUNK]
                        .rearrange("(w p) r -> p w r", w=8, p=128))
                    x_sb.append(t)

                def x_slice(i, kc):
                    h, w = divmod(i * 4 + kc, 8)
                    return x_sb[h][:, w * CHUNK:(w + 1) * CHUNK]

                # ---- forward DFT; evict as quarter-stacked tiles ----
                # T[jj, rq] at tall free slice (jj*16+rq); partition
                # (i%4)*32 + c32.  Source psum (i, ct) band rq4 covers
                # rq = ct*4 + rq4.
                tall = tpool.tile([128, 32 * CHUNK], DT, tag="tq",
                                  name="tall")
                for i in range(NBLK):
                    jj, i4 = divmod(i, 4)
                    for cp in range(2):          # ct pair (2cp, 2cp+1)
                        ps = psf.tile([128, 2 * CHUNK], F32, tag="f",
                                      name="fps")
                        for cc in range(2):
                            ct = cp * 2 + cc
                            for kc in range(4):
                                nc.tensor.matmul(
                                    ps[:, cc * CHUNK:(cc + 1) * CHUNK],
                                    fwd_sb[:, (kc * 4 + ct) * 128:
                                           (kc * 4 + ct + 1) * 128],
                                    x_slice(i, kc),
                                    start=(kc == 0), stop=(kc == 3))
                        for rq4 in range(4):
                            # bands for rq = (2cp+cc)*4 + rq4, cc in {0,1}:
                            # free slots jj*16 + cp*8 + cc*4 + rq4
                            s0 = (jj * 16 + cp * 8 + rq4) * CHUNK
                            out_ap = (tall[i4 * 32:(i4 + 1) * 32,
                                           s0:s0 + 5 * CHUNK]
                                      .rearrange("p (cc r) -> p cc r",
                                                 cc=5, r=CHUNK)[:, ::4, :])
                            nc.any.tensor_copy(
                                out=out_ap,
                                in_=ps[rq4 * 32:(rq4 + 1) * 32, :]
                                .rearrange("p (cc r) -> p cc r",
                                           cc=2, r=CHUNK))

                # ---- mid: 2 dense matmuls per (kk, rq) output tile ----
                # out partitions (o4, c'32); band o4 -> Y_sep[kk*4+o4]
                # partitions rq4*32 + c'32, free ct*CHUNK (rq = ct*4+rq4).
                y_sep = []
                for o in range(NBLK):
                    y_sep.append(ypool.tile([128, 4 * CHUNK], DT, tag="ys",
                                            name="yst"))
                for kk in range(2):
                    ust = upool.tile([128, 16 * CHUNK], DT, tag="us",
                                     name="ust")
                    for rq in range(16):
                        ps = psm.tile([128, CHUNK], F32, tag="m", name="mps")
                        for jj in range(2):
                            g = (kk * 2 + jj) * 16 + rq
                            nc.tensor.matmul(
                                ps[:], mid_sb[:, g * 128:(g + 1) * 128],
                                tall[:, (jj * 16 + rq) * CHUNK:
                                     (jj * 16 + rq + 1) * CHUNK],
                                start=(jj == 0), stop=(jj == 1))
                        nc.any.tensor_copy(
                            out=ust[:, rq * CHUNK:(rq + 1) * CHUNK],
                            in_=ps[:])
                    # scatter: ust[(o4,c32), (ct*4+rq4)*CHUNK] ->
                    #          y_sep[kk*4+o4][(rq4,c32), ct*CHUNK]
                    for o4 in range(4):
                        for rq4 in range(4):
                            nc.sync.dma_start(
                                y_sep[kk * 4 + o4]
                                [rq4 * 32:(rq4 + 1) * 32, :]
                                .rearrange("p (ct r) -> p ct r",
                                           ct=4, r=CHUNK),
                                ust[o4 * 32:(o4 + 1) * 32, :]
                                .rearrange("p (ct rq4 r) -> p ct rq4 r",
                                           ct=4, rq4=4, r=CHUNK)
                                [:, :, rq4, :])

                # ---- inverse DFT per output block o ----
                for o in range(NBLK):
                    ot = opool.tile([128, 4 * CHUNK], DT, tag="o", name="ot")
                    for tt in range(4):
                        ps = psi.tile([128, CHUNK], F32, tag="i", name="ips")
                        for ct in range(4):
                            nc.tensor.matmul(
                                ps[:],
                                inv_sb[:, (ct * 4 + tt) * 128:
                                       (ct * 4 + tt + 1) * 128],
                                y_sep[o][:, ct * CHUNK:(ct + 1) * CHUNK],
                                start=(ct == 0), stop=(ct == 3))
                        nc.any.tensor_copy(
                            out=ot[:, tt * CHUNK:(tt + 1) * CHUNK], in_=ps[:])
                    nc.sync.dma_start(
                        yT[o * P:(o + 1) * P, r0:r0 + CHUNK]
                        .rearrange("(tt p) r -> p tt r", tt=4, p=128),
                        ot[:].rearrange("p (tt r) -> p tt r", tt=4, r=CHUNK))

            def body(_=None):
                for c in range(n_chunks):
                    chunk_body(c)

            if repeat == 1:
                body()
            else:
                with tc.For_i(0, repeat, 1) as it:
                    body(it)
    nc.compile()
    _NC_CACHE[key] = nc
    return nc


def make_in_maps(x, spectral_real, spectral_imag, sign_flip):
    fwdT, invT, midQ = host_transforms(spectral_real, spectral_imag)
    xs = (x.reshape(-1, F) * sign_flip[None, :].astype(np.float32))
    in_maps = []
    for c in range(N_CORES):
        shard = xs[c * RPC:(c + 1) * RPC]
        in_maps.append({
            "xT": np.ascontiguousarray(shard.T).astype(NPDT),
            "fwdT": fwdT, "invT": invT, "midQ": midQ,
        })
    return in_maps


def kernel(x, spectral_real, spectral_imag, sign_flip, bias):
    x = np.asarray(x, np.float32)
    spectral_real = np.asarray(spectral_real, np.float32)
    spectral_imag = np.asarray(spectral_imag, np.float32)
    sign_flip = np.asarray(sign_flip, np.float32)
    bias = np.asarray(bias, np.float32)
    batch_shape = x.shape[:-1]

    in_maps = make_in_maps(x, spectral_real, spectral_imag, sign_flip)
    nc = build_nc()
    res = run_bass_kernel_spmd(nc, in_maps, list(range(N_CORES)))
    y = np.concatenate(
        [np.ascontiguousarray(np.asarray(res.results[c]["yT"],
                                         dtype=np.float32).T)
         for c in range(N_CORES)],
        axis=0)
    y = y + bias[None, :]
    return y.reshape(*batch_shape, F).astype(np.float32)


# revision 28
# speedup vs baseline: 1.2475x; 1.0164x over previous
"""nn_BlockCirculantLinear on 8 Trainium2 cores (Bass/Tile, bf16).

Math.  Per output block o (8 blocks of P=512):
    y_o = sum_i real(IFFT(Lam[o,i] * FFT(x_i * sf_i)))
With x real this factors into three real linear stages:
  1. forward  : X_i = Fe @ (sf*x)_i^T      -- real-DFT coords, per block i
  2. middle   : Y_o = sum_i M_oi X_i       -- per-frequency 2x2 mixes
  3. inverse  : y_o^T = Fi @ Y_o
Coordinate packing per block: c=0 -> (f=0, re); c=1 -> (f=256, re);
c=2f/2f+1 -> (f, re/im) for f=1..255.  Frequency pair (f, P-f) folds into
one 2x2 real block via Hermitian symmetry, so M[o,i] couples only
same-frequency coordinate pairs (2x2 block diagonal).

Key restructure vs the dense-mid formulation ("quarter stacking"): the
mid only couples same-frequency coords, so a 128x128 mid matmul with a
per-block 128-coord tile has 2/128 useful density.  Instead, stack FOUR
blocks x 32 coords per 128-partition tile: T[jj,rq][p=(i4,c32), r].
Then the mid is 2 dense accumulating matmuls per output tile
(2 kk x 16 rq x 2 jj = 64 matmuls/chunk instead of 256), and the dense
mid constant shrinks from 16.8 MiB streamed every chunk (67 MB/pass of
HBM) to a 2 MiB resident table.  The (re)stacking costs nothing extra:
PSUM evictions become partition-shifted 32-row band copies (DVE/ACT/Pool
lanes can read psum band [32k,32k+32) and write sbuf band [32m,32m+32)),
which replace the full-tile eviction copies the kernel needs anyway.

Per core per pass: (128 fwd + 64 mid + 128 inv) matmuls[K=128,M<=128,
N=512]/chunk x 4 chunks = 1280 matmuls ~= 273us PE; DMA only x+y
(33.5 MB bf16) ~= 95us; eviction copies ~49K cyc/chunk/engine over 3
engines.  All streams bf16 (tolerance 2e-2 >> bf16 error ~6e-3).

Sharding: data-parallel -- 16384 rows split 8 ways; constants replicated.
sign_flip folded into x on host; bias added on host after gathering.
"""
import os
from contextlib import ExitStack

import numpy as np

import concourse.mybir as mybir
import concourse.bacc as bacc
import concourse.tile as tile
from concourse.bass_utils import run_bass_kernel_spmd

N_CORES = 8
ROWS = 16384
RPC = ROWS // N_CORES      # 2048 rows per core
F = 4096
P = 512
NBLK = 8
CHUNK = 512                # rows per pipelined chunk (= matmul free dim)
_NC_CACHE = {}

DT = mybir.dt.bfloat16
NPDT = mybir.dt.np(DT)


def build_transforms(spectral_real, spectral_imag, dtype=np.float64):
    """Fe [c, feat], Fi [t, c], M [o, i, c_out, c_in] (2x2 block diagonal)."""
    s = np.arange(P)
    f = np.arange(1, P // 2)
    theta = 2 * np.pi * np.outer(f, s) / P

    Fe = np.zeros((P, P), dtype)
    Fe[0, :] = 1.0
    Fe[1, :] = (-1.0) ** s
    Fe[2::2, :] = np.cos(theta)
    Fe[3::2, :] = -np.sin(theta)

    Fi = np.zeros((P, P), dtype)
    Fi[:, 0] = 1.0 / P
    Fi[:, 1] = ((-1.0) ** s) / P
    Fi[:, 2::2] = np.cos(theta).T / P
    Fi[:, 3::2] = -np.sin(theta).T / P

    lam_r = spectral_real.astype(dtype)
    lam_i = spectral_imag.astype(dtype)
    M = np.zeros((NBLK, NBLK, P, P), dtype)
    M[:, :, 0, 0] = lam_r[:, :, 0]
    M[:, :, 1, 1] = lam_r[:, :, P // 2]
    l1r = lam_r[:, :, 1:P // 2]; l1i = lam_i[:, :, 1:P // 2]
    l2r = lam_r[:, :, :P // 2:-1]; l2i = lam_i[:, :, :P // 2:-1]
    ce = np.arange(2, P, 2); co = ce + 1
    M[:, :, ce, ce] = l1r + l2r
    M[:, :, ce, co] = l2i - l1i
    M[:, :, co, ce] = l1i - l2i
    M[:, :, co, co] = l1r + l2r
    return Fe, Fi, M


def host_transforms(spectral_real, spectral_imag):
    Fe, Fi, M = build_transforms(spectral_real, spectral_imag)
    fwdT = np.ascontiguousarray(Fe.T).astype(NPDT)           # lhsT [feat, c]
    invT = np.ascontiguousarray(Fi.T).astype(NPDT)           # lhsT [c, t]
    # quarter-stacked mid lhsT per (kk, jj, rq):
    #   lhsT[p_in=(i4,c32), p_out=(o4,c'32)] = M[kk*4+o4, jj*4+i4, c', c]
    midQ = np.zeros((2, 2, 16, 128, 128), np.float64)
    for kk in range(2):
        for jj in range(2):
            for rq in range(16):
                blk = M[kk * 4:(kk + 1) * 4, jj * 4:(jj + 1) * 4,
                        rq * 32:(rq + 1) * 32, rq * 32:(rq + 1) * 32]
                midQ[kk, jj, rq] = (blk.transpose(1, 3, 0, 2)
                                    .reshape(128, 128))
    return fwdT, invT, midQ.reshape(64, 128, 128).astype(NPDT)


def build_nc(repeat: int = 1):
    key = (CHUNK, repeat)
    if key in _NC_CACHE:
        return _NC_CACHE[key]
    nc = bacc.Bacc("TRN2", target_bir_lowering=False, debug=False,
                   num_devices=N_CORES)
    xT = nc.dram_tensor("xT", [F, RPC], DT, kind="ExternalInput")
    fwdT = nc.dram_tensor("fwdT", [P, P], DT, kind="ExternalInput")
    invT = nc.dram_tensor("invT", [P, P], DT, kind="ExternalInput")
    midQ = nc.dram_tensor("midQ", [64, 128, 128], DT, kind="ExternalInput")
    yT = nc.dram_tensor("yT", [F, RPC], DT, kind="ExternalOutput")

    n_chunks = RPC // CHUNK
    F32 = mybir.dt.float32

    with tile.TileContext(nc) as tc:
        with ExitStack() as ctx:
            const = ctx.enter_context(tc.tile_pool(name="const", bufs=1))
            fwd_sb = const.tile([128, 16 * 128], DT)
            inv_sb = const.tile([128, 16 * 128], DT)
            mid_sb = const.tile([128, 64 * 128], DT)
            # constants: single batched DMA each, partition-outermost APs
            for sb, dr in ((fwd_sb, fwdT), (inv_sb, invT)):
                nc.sync.dma_start(
                    sb[:].rearrange("p (kc m) -> p kc m", kc=4, m=512),
                    dr[:, :].rearrange("(kc p) m -> p kc m", kc=4, p=128))
            nc.sync.dma_start(
                mid_sb[:].rearrange("p (g m) -> p g m", g=64, m=128),
                midQ[:, :, :].rearrange("g p m -> p g m"))

            copy_fns = [nc.vector.tensor_copy, nc.scalar.copy]
            copy_ctr = [0]

            def rr_copy(out, in_):
                f = copy_fns[copy_ctr[0] % 2]
                copy_ctr[0] += 1
                f(out=out, in_=in_)

            xpool = ctx.enter_context(tc.tile_pool(name="x", bufs=4))
            tpool = ctx.enter_context(tc.tile_pool(name="tq", bufs=2))
            upool = ctx.enter_context(tc.tile_pool(name="us", bufs=2))
            ypool = ctx.enter_context(tc.tile_pool(name="ys", bufs=11))
            opool = ctx.enter_context(tc.tile_pool(name="out", bufs=2))
            psf = ctx.enter_context(tc.tile_pool(name="psf", bufs=2,
                                                 space="PSUM"))
            psm = ctx.enter_context(tc.tile_pool(name="psm", bufs=2,
                                                 space="PSUM"))
            psi = ctx.enter_context(tc.tile_pool(name="psi", bufs=2,
                                                 space="PSUM"))

            state = {}   # per-chunk tiles: x_sb, tall, y_sep

            def emit_xload(c):
                r0 = c * CHUNK
                x_sb = []
                for h in range(NBLK // 2):
                    t = xpool.tile([128, 8 * CHUNK], DT, tag="x", name="xt")
                    nc.sync.dma_start(
                        t[:].rearrange("p (w r) -> p w r", w=8, r=CHUNK),
                        xT[h * 2 * P:(h + 1) * 2 * P, r0:r0 + CHUNK]
                        .rearrange("(w p) r -> p w r", w=8, p=128))
                    x_sb.append(t)
                state[c] = {"x": x_sb}

            def emit_fwd_half(c, i, cp):
                # block i, ct-pair (2cp, 2cp+1): 8 matmuls into a 2-bank
                # psum tile, then 4 double-width band copies into tall
                st = state[c]
                if "tall" not in st:
                    st["tall"] = tpool.tile([128, 32 * CHUNK], DT, tag="tq",
                                            name="tall")
                tall, x_sb = st["tall"], st["x"]
                jj, i4 = divmod(i, 4)
                ps = psf.tile([128, 2 * CHUNK], F32, tag="f", name="fps")
                for cc in range(2):
                    ct = cp * 2 + cc
                    for kc in range(4):
                        h, w = divmod(i * 4 + kc, 8)
                        nc.tensor.matmul(
                            ps[:, cc * CHUNK:(cc + 1) * CHUNK],
                            fwd_sb[:, (kc * 4 + ct) * 128:
                                   (kc * 4 + ct + 1) * 128],
                            x_sb[h][:, w * CHUNK:(w + 1) * CHUNK],
                            start=(kc == 0), stop=(kc == 3))
                for rq4 in range(4):
                    # bands for rq = (2cp+cc)*4 + rq4, cc in {0,1}:
                    # tall slots jj*16 + cp*8 + cc*4 + rq4
                    s0 = (jj * 16 + cp * 8 + rq4) * CHUNK
                    rr_copy(
                        tall[i4 * 32:(i4 + 1) * 32, s0:s0 + 5 * CHUNK]
                        .rearrange("p (cc r) -> p cc r", cc=5, r=CHUNK)
                        [:, ::4, :],
                        ps[rq4 * 32:(rq4 + 1) * 32, :]
                        .rearrange("p (cc r) -> p cc r", cc=2, r=CHUNK))

            def emit_mid_rq(c, kk, rq):
                st = state[c]
                if "ys" not in st:
                    st["ys"] = [ypool.tile([128, 4 * CHUNK], DT, tag="ys",
                                           name="yst")
                                for _ in range(NBLK)]
                if kk not in st.setdefault("ust", {}):
                    st["ust"][kk] = upool.tile([128, 16 * CHUNK], DT,
                                               tag="us", name="ust")
                tall, ust = st["tall"], st["ust"][kk]
                ps = psm.tile([128, CHUNK], F32, tag="m", name="mps")
                for jj in range(2):
                    g = (kk * 2 + jj) * 16 + rq
                    nc.tensor.matmul(
                        ps[:], mid_sb[:, g * 128:(g + 1) * 128],
                        tall[:, (jj * 16 + rq) * CHUNK:
                             (jj * 16 + rq + 1) * CHUNK],
                        start=(jj == 0), stop=(jj == 1))
                rr_copy(ust[:, rq * CHUNK:(rq + 1) * CHUNK], ps[:])

            def emit_scat(c, kk, rq4):
                st = state[c]
                ust, y_sep = st["ust"][kk], st["ys"]
                for o4 in range(4):
                    nc.sync.dma_start(
                        y_sep[kk * 4 + o4]
                        [rq4 * 32:(rq4 + 1) * 32, :]
                        .rearrange("p (ct r) -> p ct r", ct=4, r=CHUNK),
                        ust[o4 * 32:(o4 + 1) * 32, :]
                        .rearrange("p (ct rq4 r) -> p ct rq4 r",
                                   ct=4, rq4=4, r=CHUNK)[:, :, rq4, :])

            def emit_inv_quad(c, o, tt):
                st = state[c]
                if "ot" not in st:
                    st["ot"] = {}
                if o not in st["ot"]:
                    st["ot"][o] = opool.tile([128, 4 * CHUNK], DT, tag="o",
                                             name="ot")
                ot, y_sep = st["ot"][o], st["ys"]
                ps = psi.tile([128, CHUNK], F32, tag="i", name="ips")
                for ct in range(4):
                    nc.tensor.matmul(
                        ps[:],
                        inv_sb[:, (ct * 4 + tt) * 128:
                               (ct * 4 + tt + 1) * 128],
                        y_sep[o][:, ct * CHUNK:(ct + 1) * CHUNK],
                        start=(ct == 0), stop=(ct == 3))
                rr_copy(ot[:, tt * CHUNK:(tt + 1) * CHUNK], ps[:])
                if tt == 3:
                    r0 = c * CHUNK
                    nc.sync.dma_start(
                        yT[o * P:(o + 1) * P, r0:r0 + CHUNK]
                        .rearrange("(tt p) r -> p tt r", tt=4, p=128),
                        ot[:].rearrange("p (tt r) -> p tt r",
                                        tt=4, r=CHUNK))

            def body(wrap=False, _=None):
                # software pipeline, quad-granular: mid/inv of chunk c
                # interleaved ~2:1 with fwd of chunk c+1 so the PE queue
                # always holds independent matmuls to cover evict drains.
                # wrap=True (hardware repeat loop): chunk 3 interleaves the
                # NEXT ITERATION's chunk-0 x-load+fwd (pool slot rotation
                # lines up: tall 1/chunk x bufs 2, x 4/chunk x bufs 4).
                for c in range(n_chunks):
                    A = []
                    for kk in range(2):
                        for rq in range(16):
                            A.append(lambda c=c, kk=kk, rq=rq:
                                     emit_mid_rq(c, kk, rq))
                            if rq >= 12:
                                A.append(lambda c=c, kk=kk, rq4=rq - 12:
                                         emit_scat(c, kk, rq4))
                        for o4 in range(2):
                            o = kk * 4 + o4
                            for tt in range(4):
                                A.append(lambda c=c, o=o, tt=tt:
                                         emit_inv_quad(c, o, tt))
                    for o4 in range(2, 4):
                        for kk in range(2):
                            o = kk * 4 + o4
                            for tt in range(4):
                                A.append(lambda c=c, o=o, tt=tt:
                                         emit_inv_quad(c, o, tt))
                    nxt = c + 1
                    if nxt < n_chunks or wrap:
                        nxt %= n_chunks
                        B = [lambda nxt=nxt: emit_xload(nxt)]
                        for i in range(NBLK):
                            for cp in range(2):
                                B.append(lambda nxt=nxt, i=i, cp=cp:
                                         emit_fwd_half(nxt, i, cp))
                    else:
                        B = []
                    # interleave: ~5 A units per B unit (|A|=80, |B|=17)
                    ai = bi = 0
                    while ai < len(A) or bi < len(B):
                        for _ in range(4):
                            if ai < len(A):
                                A[ai]()
                                ai += 1
                        if bi < len(B):
                            B[bi]()
                            bi += 1

            emit_xload(0)
            for i in range(NBLK):
                for cp in range(2):
                    emit_fwd_half(0, i, cp)
            if repeat == 1:
                body(wrap=False)
            elif os.environ.get("SIM_UNROLL"):
                for _ in range(repeat):
                    body(wrap=True)
            else:
                with tc.For_i(0, repeat, 1) as it:
                    body(wrap=True, _=it)
    nc.compile()
    _NC_CACHE[key] = nc
    return nc


def make_in_maps(x, spectral_real, spectral_imag, sign_flip):
    fwdT, invT, midQ = host_transforms(spectral_real, spectral_imag)
    xs = (x.reshape(-1, F) * sign_flip[None, :].astype(np.float32))
    in_maps = []
    for c in range(N_CORES):
        shard = xs[c * RPC:(c + 1) * RPC]
        in_maps.append({
            "xT": np.ascontiguousarray(shard.T).astype(NPDT),
            "fwdT": fwdT, "invT": invT, "midQ": midQ,
        })
    return in_maps


def kernel(x, spectral_real, spectral_imag, sign_flip, bias):
    x = np.asarray(x, np.float32)
    spectral_real = np.asarray(spectral_real, np.float32)
    spectral_imag = np.asarray(spectral_imag, np.float32)
    sign_flip = np.asarray(sign_flip, np.float32)
    bias = np.asarray(bias, np.float32)
    batch_shape = x.shape[:-1]

    in_maps = make_in_maps(x, spectral_real, spectral_imag, sign_flip)
    nc = build_nc()
    res = run_bass_kernel_spmd(nc, in_maps, list(range(N_CORES)))
    y = np.concatenate(
        [np.ascontiguousarray(np.asarray(res.results[c]["yT"],
                                         dtype=np.float32).T)
         for c in range(N_CORES)],
        axis=0)
    y = y + bias[None, :]
    return y.reshape(*batch_shape, F).astype(np.float32)
